# revision 20
# baseline (speedup 1.0000x reference)
"""ReEig (eigenvalue clamp + reconstruct) Trainium2 Bass kernel, v4.

rec = V @ diag(max(lam, eps)) @ V^T for 8192 symmetric 64x64 fp32 matrices,
via a tuned Newton-Schulz matrix-sign iteration in bf16 on the PE
(rec = 0.5*(X + |X|); see kernel_baseline.py for the derivation).

Structure (vs the 438us baseline):

1. PE packing of the X-weighted phases. A matmul costs LDWEIGHTS
   (stationary cols) + MATMUL (moving rows) on the PE; per-matrix 64x64
   matmuls stream only 64 of 128 partitions. A full-array matmul with
   BLOCK-DIAGONAL weights diag(X_m1, X_m2) computes both matrices of a
   partition-pair in one 64-beat stream. Block-diag weights are free only
   for the input X (the input DMA writes them; off-blocks zeroed once), so
   the three X-weighted phases are packed: Y0 = X^T X, Z0 = X^T Yp0, and
   the final W = X^T P~. Middle iterations keep quadrant matmuls (building
   block-diag P_k tiles costs more than it saves on every path: engine
   copies are half-width = full-time, DMA builds pay ~0.6-1us descriptor
   issue each).

2. Big blocks: 32 matrices per block (16 partition-pairs) halve every DMA
   count and per-instruction overhead; psum tiles span 2 banks (pool of 4).

3. bf16 I/O: host pre-casts X to bf16 (device only ever consumed bf16(X)),
   output DMA writes bf16. Halves both DMA directions, frees gpsimd from
   SWDGE casts, and moves the fp32 cast to the host.

4. Engine spreading: Yp copies on Act, P' STTs on DVE except one iteration
   on Pool, input DMAs issued from SP, output DMAs from Pool. No absorber
   DMAs (excess sem waits go to NOP-splits).

Sharding: embarrassingly parallel over batch; 1024 matrices/core.
"""

import numpy as np

B, N = 8192, 64
N_CORES = 8
B_SHARD = B // N_CORES  # 1024
GH = 16                 # matrix pairs per block
G = 2 * GH              # 32 matrices per block
ILEAVE = 6              # blocks interleaved phase-by-phase
PF_WAVES = 2            # input prefetch distance, in waves
NSLOT = (PF_WAVES + 1) * ILEAVE + 2  # in-flight input slots

S = 15.299060624329034
C = 1.0130927931015137
SCHED = [
    (2.5095738631314206, 2.734605534291715),
    (2.425522948311836, 2.2319801608079994),
    (2.251838491935489, 1.333974101194705),
    (1.430977959043743, 0.44208718303333105),
]


def _split_excess_waits(nc):
    """Instructions have one HW sync-wait slot; Tile's slot-release logic
    can emit more. Move the excess onto nofuse NOPs just before the
    instruction on the same engine."""
    import concourse.mybir as mybir

    max_waits = 1
    n_nops = 0
    for fn in nc.m.functions:
        for bb in fn.blocks:
            out = []
            for inst in bb.instructions:
                si = inst.sync_info
                if si is not None and len(si.on_wait) > max_waits:
                    waits = list(si.on_wait)
                    excess, keep = waits[:-max_waits], waits[-max_waits:]
                    while excess:
                        chunk, excess = excess[:max_waits], excess[max_waits:]
                        nop = mybir.InstNoOp(
                            name=f"{inst.name}-wsplit{n_nops}",
                            engine=inst.engine,
                            sync_info=mybir.SyncInfo(on_wait=chunk, on_update=[]),
                            bass_nofuse=True,
                        )
                        n_nops += 1
                        nc.inst_map[nop.name] = nop
                        out.append(nop)
                    inst.sync_info = mybir.SyncInfo(
                        on_wait=keep, on_update=list(si.on_update)
                    )
                out.append(inst)
            bb.instructions[:] = out
    return n_nops


def build_bass(b_shard=B_SHARD):
    import concourse.bass as bass
    import concourse.mybir as mybir
    import concourse.tile as tile

    f32 = mybir.dt.float32
    bf16 = mybir.dt.bfloat16
    Alu = mybir.AluOpType

    K = len(SCHED)
    nblk = b_shard // G
    nc = bass.Bass(name="reeig")
    x = nc.dram_tensor("x", [b_shard, N, N], bf16, kind="ExternalInput")
    out = nc.dram_tensor("out", [b_shard, N, N], bf16, kind="ExternalOutput")

    QUAD = ((0, (0, 0)), (64, (64, 64)))

    with tile.TileContext(nc) as tc:
        with (
            tc.tile_pool(name="const", bufs=1) as cpool,
            tc.tile_pool(name="data", bufs=ILEAVE + 3) as dpool,
            tc.tile_pool(name="xin", bufs=NSLOT) as xpool,
            tc.tile_pool(name="psum", bufs=4, space="PSUM") as ppool,
        ):
            # Block-diagonal X weight slots: one big persistent tile,
            # manually rotated; off-diagonal blocks zeroed once (input DMAs
            # only touch diagonal blocks), so every [128, j, 128] slice
            # stays diag(X_m1, X_m2).
            ablk = cpool.tile([128, NSLOT, GH, 2 * N], bf16, tag="ablk")

            at_tiles = {}

            def issue_load(b):
                if b >= nblk or b in at_tiles:
                    return
                m0 = b * G
                at = xpool.tile([128, GH, N], bf16, tag="X")
                s = b % NSLOT
                nc.sync.dma_start(
                    ablk[0:64, s, :, 0:N],
                    x[m0 : m0 + GH].rearrange("g r c -> r g c"),
                )
                nc.sync.dma_start(
                    ablk[64:128, s, :, N : 2 * N],
                    x[m0 + GH : m0 + G].rearrange("g r c -> r g c"),
                )
                nc.sync.dma_start(
                    at[0:64], x[m0 : m0 + GH].rearrange("g r c -> r g c")
                )
                nc.sync.dma_start(
                    at[64:128], x[m0 + GH : m0 + G].rearrange("g r c -> r g c")
                )
                at_tiles[b] = at

            def packed_mm(dst, rhs_t, slot):
                for j in range(GH):
                    nc.tensor.matmul(
                        dst[:, j],
                        lhsT=ablk[:, slot, j],
                        rhs=rhs_t[:, j],
                        start=True, stop=True,
                    )

            def quad_mm(dst, lhs_t, rhs_t):
                for j in range(GH):
                    for lo, tp in QUAD:
                        nc.tensor.matmul(
                            dst[lo : lo + 64, j],
                            lhsT=lhs_t[lo : lo + 64, j],
                            rhs=rhs_t[lo : lo + 64, j],
                            start=True, stop=True, tile_position=tp,
                        )

            # zero the ablk slots needed first, then interleave the rest
            # with the initial prefetch loads
            for s in range(ILEAVE):
                nc.gpsimd.memset(ablk[:, s], 0.0)
            for b in range(ILEAVE):
                issue_load(b)
            for s in range(ILEAVE, NSLOT):
                nc.gpsimd.memset(ablk[:, s], 0.0)
            for b in range(ILEAVE, PF_WAVES * ILEAVE):
                issue_load(b)
            for bp in range(0, nblk, ILEAVE):
                blocks = [b for b in range(bp, min(bp + ILEAVE, nblk))]
                pf = [bp + PF_WAVES * ILEAVE + i for i in range(ILEAVE)]
                st = {}
                for b in blocks:
                    st[b] = {"at": at_tiles.pop(b)}

                for k, (ca, cb) in enumerate(SCHED):
                    g = C / 2 if k == K - 1 else 1.0
                    ys = 1.0 / S**3 if k == 0 else 1.0
                    ps = 1.0 / S if k == 0 else 1.0
                    for i, b in enumerate(blocks):
                        s = st[b]
                        src_t = s["at"] if k == 0 else s["pt"]
                        yt = ppool.tile([128, GH, N], f32, tag="PS")
                        if k == 0:
                            packed_mm(yt, src_t, b % NSLOT)
                        else:
                            quad_mm(yt, src_t, src_t)
                        s["yt"] = yt
                        if i < len(pf) and i % K == k:
                            issue_load(pf[i])
                    for b in blocks:
                        s = st[b]
                        ypt = dpool.tile([128, GH, N], bf16, tag="Yp")
                        nc.scalar.mul(ypt[:], s["yt"][:], -cb * g * ys)
                        s["ypt"] = ypt
                    for b in blocks:
                        s = st[b]
                        src_t = s["at"] if k == 0 else s["pt"]
                        zt = ppool.tile([128, GH, N], f32, tag="PS")
                        if k == 0:
                            packed_mm(zt, s["ypt"], b % NSLOT)
                        else:
                            quad_mm(zt, src_t, s["ypt"])
                        s["zt"] = zt
                    for b in blocks:
                        s = st[b]
                        src_t = s["at"] if k == 0 else s["pt"]
                        pt = dpool.tile([128, GH, N], bf16, tag="P")
                        nc.vector.scalar_tensor_tensor(
                            out=pt[:], in0=src_t[:], scalar=ca * g * ps,
                            in1=s["zt"][:], op0=Alu.mult, op1=Alu.add,
                        )
                        s["pt"] = pt

                for b in blocks:
                    s = st[b]
                    wt = ppool.tile([128, GH, N], f32, tag="PS")
                    packed_mm(wt, s["pt"], b % NSLOT)
                    s["wt"] = wt
                    rt = dpool.tile([128, GH, N], bf16, tag="R")
                    # W = (C*s/2) A P~ only; the 0.5*X half of rec is added
                    # on the host in fp32 (cheaper here and more accurate)
                    nc.scalar.mul(rt[:], s["wt"][:], 1.0)
                    m0 = b * G
                    nc.gpsimd.dma_start(
                        out[m0 : m0 + GH].rearrange("g r c -> r g c"), rt[0:64]
                    )
                    nc.gpsimd.dma_start(
                        out[m0 + GH : m0 + G].rearrange("g r c -> r g c"),
                        rt[64:128],
                    )

    _split_excess_waits(nc)
    return nc


_CACHE = {}


def run(x: np.ndarray, **spmd_kwargs):
    import ml_dtypes
    from concourse.bass_utils import run_bass_kernel_spmd

    assert x.shape == (B, N, N) and x.dtype == np.float32
    if "nc" not in _CACHE:
        _CACHE["nc"] = build_bass()
    nc = _CACHE["nc"]
    xb = x.astype(ml_dtypes.bfloat16)
    shards = xb.reshape(N_CORES, B_SHARD, N, N)
    in_maps = [{"x": np.ascontiguousarray(shards[i])} for i in range(N_CORES)]
    return run_bass_kernel_spmd(
        nc, in_maps, core_ids=list(range(N_CORES)), **spmd_kwargs
    )


def kernel(x: np.ndarray) -> np.ndarray:
    x = np.ascontiguousarray(np.asarray(x), dtype=np.float32)
    res = run(x)
    w = np.concatenate(
        [r["out"].astype(np.float32) for r in res.results], axis=0
    )
    # rec = 0.5*X + W, symmetrized (W is symmetric up to bf16 matmul noise;
    # averaging with the transpose halves it)
    out = 0.5 * x + 0.5 * (w + w.transpose(0, 2, 1))
    return out.astype(np.float32)


# revision 21
# speedup vs baseline: 1.0119x; 1.0119x over previous
"""ReEig (eigenvalue clamp + reconstruct) Trainium2 Bass kernel, v4.

rec = V @ diag(max(lam, eps)) @ V^T for 8192 symmetric 64x64 fp32 matrices,
via a tuned Newton-Schulz matrix-sign iteration in bf16 on the PE
(rec = 0.5*(X + |X|); see kernel_baseline.py for the derivation).

Structure (vs the 438us baseline):

1. PE packing of the X-weighted phases. A matmul costs LDWEIGHTS
   (stationary cols) + MATMUL (moving rows) on the PE; per-matrix 64x64
   matmuls stream only 64 of 128 partitions. A full-array matmul with
   BLOCK-DIAGONAL weights diag(X_m1, X_m2) computes both matrices of a
   partition-pair in one 64-beat stream. Block-diag weights are free only
   for the input X (the input DMA writes them; off-blocks zeroed once), so
   the three X-weighted phases are packed: Y0 = X^T X, Z0 = X^T Yp0, and
   the final W = X^T P~. Middle iterations keep quadrant matmuls (building
   block-diag P_k tiles costs more than it saves on every path: engine
   copies are half-width = full-time, DMA builds pay ~0.6-1us descriptor
   issue each).

2. Big blocks: 32 matrices per block (16 partition-pairs) halve every DMA
   count and per-instruction overhead; psum tiles span 2 banks (pool of 4).

3. bf16 I/O: host pre-casts X to bf16 (device only ever consumed bf16(X)),
   output DMA writes bf16. Halves both DMA directions, frees gpsimd from
   SWDGE casts, and moves the fp32 cast to the host.

4. Engine spreading: Yp copies on Act, P' STTs on DVE except one iteration
   on Pool, input DMAs issued from SP, output DMAs from Pool. No absorber
   DMAs (excess sem waits go to NOP-splits).

Sharding: embarrassingly parallel over batch; 1024 matrices/core.
"""

import numpy as np

B, N = 8192, 64
N_CORES = 8
B_SHARD = B // N_CORES  # 1024
GH = 16                 # matrix pairs per block
G = 2 * GH              # 32 matrices per block
ILEAVE = 6              # blocks interleaved phase-by-phase
PF_WAVES = 2            # input prefetch distance, in waves
NSLOT = (PF_WAVES + 1) * ILEAVE + 2  # in-flight input slots

S = 15.299060624329034
C = 1.0130927931015137
SCHED = [
    (2.5095738631314206, 2.734605534291715),
    (2.425522948311836, 2.2319801608079994),
    (2.251838491935489, 1.333974101194705),
    (1.430977959043743, 0.44208718303333105),
]


def _split_excess_waits(nc):
    """Instructions have one HW sync-wait slot; Tile's slot-release logic
    can emit more. Move the excess onto nofuse NOPs just before the
    instruction on the same engine."""
    import concourse.mybir as mybir

    max_waits = 1
    n_nops = 0
    for fn in nc.m.functions:
        for bb in fn.blocks:
            out = []
            for inst in bb.instructions:
                si = inst.sync_info
                if si is not None and len(si.on_wait) > max_waits:
                    waits = list(si.on_wait)
                    excess, keep = waits[:-max_waits], waits[-max_waits:]
                    while excess:
                        chunk, excess = excess[:max_waits], excess[max_waits:]
                        nop = mybir.InstNoOp(
                            name=f"{inst.name}-wsplit{n_nops}",
                            engine=inst.engine,
                            sync_info=mybir.SyncInfo(on_wait=chunk, on_update=[]),
                            bass_nofuse=True,
                        )
                        n_nops += 1
                        nc.inst_map[nop.name] = nop
                        out.append(nop)
                    inst.sync_info = mybir.SyncInfo(
                        on_wait=keep, on_update=list(si.on_update)
                    )
                out.append(inst)
            bb.instructions[:] = out
    return n_nops


def build_bass(b_shard=B_SHARD):
    import concourse.bass as bass
    import concourse.mybir as mybir
    import concourse.tile as tile

    f32 = mybir.dt.float32
    bf16 = mybir.dt.bfloat16
    Alu = mybir.AluOpType

    K = len(SCHED)
    nblk = b_shard // G
    nc = bass.Bass(name="reeig")
    x = nc.dram_tensor("x", [b_shard, N, N], bf16, kind="ExternalInput")
    out = nc.dram_tensor("out", [b_shard, N, N], bf16, kind="ExternalOutput")

    QUAD = ((0, (0, 0)), (64, (64, 64)))

    with tile.TileContext(nc) as tc:
        with (
            tc.tile_pool(name="const", bufs=1) as cpool,
            tc.tile_pool(name="data", bufs=ILEAVE + 3) as dpool,
            tc.tile_pool(name="xin", bufs=NSLOT) as xpool,
            tc.tile_pool(name="psum", bufs=4, space="PSUM") as ppool,
        ):
            # Block-diagonal X weight slots: one big persistent tile,
            # manually rotated; off-diagonal blocks zeroed once (input DMAs
            # only touch diagonal blocks), so every [128, j, 128] slice
            # stays diag(X_m1, X_m2).
            ablk = cpool.tile([128, NSLOT, GH, 2 * N], bf16, tag="ablk")

            at_tiles = {}

            def issue_load(b):
                if b >= nblk or b in at_tiles:
                    return
                m0 = b * G
                at = xpool.tile([128, GH, N], bf16, tag="X")
                s = b % NSLOT
                nc.sync.dma_start(
                    ablk[0:64, s, :, 0:N],
                    x[m0 : m0 + GH].rearrange("g r c -> r g c"),
                )
                nc.sync.dma_start(
                    ablk[64:128, s, :, N : 2 * N],
                    x[m0 + GH : m0 + G].rearrange("g r c -> r g c"),
                )
                nc.sync.dma_start(
                    at[0:64], x[m0 : m0 + GH].rearrange("g r c -> r g c")
                )
                nc.sync.dma_start(
                    at[64:128], x[m0 + GH : m0 + G].rearrange("g r c -> r g c")
                )
                at_tiles[b] = at

            def packed_mm(dst, rhs_t, slot):
                for j in range(GH):
                    nc.tensor.matmul(
                        dst[:, j],
                        lhsT=ablk[:, slot, j],
                        rhs=rhs_t[:, j],
                        start=True, stop=True,
                    )

            def quad_mm(dst, lhs_t, rhs_t):
                for j in range(GH):
                    for lo, tp in QUAD:
                        nc.tensor.matmul(
                            dst[lo : lo + 64, j],
                            lhsT=lhs_t[lo : lo + 64, j],
                            rhs=rhs_t[lo : lo + 64, j],
                            start=True, stop=True, tile_position=tp,
                        )

            # zero the ablk slots needed first, then interleave the rest
            # with the initial prefetch loads
            for s in range(ILEAVE):
                nc.gpsimd.memset(ablk[:, s], 0.0)
            for b in range(ILEAVE):
                issue_load(b)
            for s in range(ILEAVE, NSLOT):
                nc.gpsimd.memset(ablk[:, s], 0.0)
            for b in range(ILEAVE, PF_WAVES * ILEAVE):
                issue_load(b)
            for bp in range(0, nblk, ILEAVE):
                blocks = [b for b in range(bp, min(bp + ILEAVE, nblk))]
                pf = [bp + PF_WAVES * ILEAVE + i for i in range(ILEAVE)]
                st = {}
                for b in blocks:
                    st[b] = {"at": at_tiles.pop(b)}

                for k, (ca, cb) in enumerate(SCHED):
                    g = C / 2 if k == K - 1 else 1.0
                    ys = 1.0 / S**3 if k == 0 else 1.0
                    ps = 1.0 / S if k == 0 else 1.0
                    for i, b in enumerate(blocks):
                        s = st[b]
                        src_t = s["at"] if k == 0 else s["pt"]
                        yt = ppool.tile([128, GH, N], f32, tag="PS")
                        if k == 0:
                            packed_mm(yt, src_t, b % NSLOT)
                        else:
                            quad_mm(yt, src_t, src_t)
                        s["yt"] = yt
                        if i < len(pf) and i % K == k:
                            issue_load(pf[i])
                    for b in blocks:
                        s = st[b]
                        ypt = dpool.tile([128, GH, N], bf16, tag="Yp")
                        nc.scalar.mul(ypt[:], s["yt"][:], -cb * g * ys)
                        s["ypt"] = ypt
                    for b in blocks:
                        s = st[b]
                        src_t = s["at"] if k == 0 else s["pt"]
                        zt = ppool.tile([128, GH, N], f32, tag="PS")
                        if k == 0:
                            packed_mm(zt, s["ypt"], b % NSLOT)
                        else:
                            quad_mm(zt, src_t, s["ypt"])
                        s["zt"] = zt
                    for b in blocks:
                        s = st[b]
                        src_t = s["at"] if k == 0 else s["pt"]
                        pt = dpool.tile([128, GH, N], bf16, tag="P")
                        nc.vector.scalar_tensor_tensor(
                            out=pt[:], in0=src_t[:], scalar=ca * g * ps,
                            in1=s["zt"][:], op0=Alu.mult, op1=Alu.add,
                        )
                        s["pt"] = pt

                for b in blocks:
                    s = st[b]
                    wt = ppool.tile([128, GH, N], f32, tag="PS")
                    packed_mm(wt, s["pt"], b % NSLOT)
                    s["wt"] = wt
                    rt = dpool.tile([128, GH, N], bf16, tag="R")
                    # W = (C*s/2) A P~ only; the 0.5*X half of rec is added
                    # on the host in fp32 (cheaper here and more accurate).
                    # Alternate the psum drain between Act and DVE so neither
                    # queue gates the next wave's Y0 psum reuse.
                    if b % 2 == 0:
                        nc.scalar.mul(rt[:], s["wt"][:], 1.0)
                    else:
                        nc.vector.tensor_scalar_mul(rt[:], s["wt"][:], 1.0)
                    m0 = b * G
                    nc.gpsimd.dma_start(
                        out[m0 : m0 + GH].rearrange("g r c -> r g c"), rt[0:64]
                    )
                    nc.gpsimd.dma_start(
                        out[m0 + GH : m0 + G].rearrange("g r c -> r g c"),
                        rt[64:128],
                    )

    _split_excess_waits(nc)
    return nc


_CACHE = {}


def run(x: np.ndarray, **spmd_kwargs):
    import ml_dtypes
    from concourse.bass_utils import run_bass_kernel_spmd

    assert x.shape == (B, N, N) and x.dtype == np.float32
    if "nc" not in _CACHE:
        _CACHE["nc"] = build_bass()
    nc = _CACHE["nc"]
    xb = x.astype(ml_dtypes.bfloat16)
    shards = xb.reshape(N_CORES, B_SHARD, N, N)
    in_maps = [{"x": np.ascontiguousarray(shards[i])} for i in range(N_CORES)]
    return run_bass_kernel_spmd(
        nc, in_maps, core_ids=list(range(N_CORES)), **spmd_kwargs
    )


def kernel(x: np.ndarray) -> np.ndarray:
    x = np.ascontiguousarray(np.asarray(x), dtype=np.float32)
    res = run(x)
    w = np.concatenate(
        [r["out"].astype(np.float32) for r in res.results], axis=0
    )
    # rec = 0.5*X + W, symmetrized (W is symmetric up to bf16 matmul noise;
    # averaging with the transpose halves it)
    out = 0.5 * x + 0.5 * (w + w.transpose(0, 2, 1))
    return out.astype(np.float32)


# revision 25
# speedup vs baseline: 1.0294x; 1.0172x over previous
"""ReEig (eigenvalue clamp + reconstruct) Trainium2 Bass kernel, v4.

rec = V @ diag(max(lam, eps)) @ V^T for 8192 symmetric 64x64 fp32 matrices,
via a tuned Newton-Schulz matrix-sign iteration in bf16 on the PE
(rec = 0.5*(X + |X|); see kernel_baseline.py for the derivation).

Structure (vs the 438us baseline):

1. PE packing of the X-weighted phases. A matmul costs LDWEIGHTS
   (stationary cols) + MATMUL (moving rows) on the PE; per-matrix 64x64
   matmuls stream only 64 of 128 partitions. A full-array matmul with
   BLOCK-DIAGONAL weights diag(X_m1, X_m2) computes both matrices of a
   partition-pair in one 64-beat stream. Block-diag weights are free only
   for the input X (the input DMA writes them; off-blocks zeroed once), so
   the three X-weighted phases are packed: Y0 = X^T X, Z0 = X^T Yp0, and
   the final W = X^T P~. Middle iterations keep quadrant matmuls (building
   block-diag P_k tiles costs more than it saves on every path: engine
   copies are half-width = full-time, DMA builds pay ~0.6-1us descriptor
   issue each).

2. Big blocks: 32 matrices per block (16 partition-pairs) halve every DMA
   count and per-instruction overhead; psum tiles span 2 banks (pool of 4).

3. bf16 I/O: host pre-casts X to bf16 (device only ever consumed bf16(X)),
   output DMA writes bf16. Halves both DMA directions, frees gpsimd from
   SWDGE casts, and moves the fp32 cast to the host.

4. Engine spreading: Yp copies on Act, P' STTs on DVE except one iteration
   on Pool, input DMAs issued from SP, output DMAs from Pool. No absorber
   DMAs (excess sem waits go to NOP-splits).

Sharding: embarrassingly parallel over batch; 1024 matrices/core.
"""

import numpy as np

B, N = 8192, 64
N_CORES = 8
B_SHARD = B // N_CORES  # 1024
GH = 16                 # matrix pairs per block
G = 2 * GH              # 32 matrices per block
ILEAVE = 6              # blocks interleaved phase-by-phase
PF_WAVES = 2            # input prefetch distance, in waves
NSLOT = (PF_WAVES + 1) * ILEAVE + 2  # in-flight input slots

S = 15.299060624329034
C = 1.0130927931015137
SCHED = [
    (2.5095738631314206, 2.734605534291715),
    (2.425522948311836, 2.2319801608079994),
    (2.251838491935489, 1.333974101194705),
    (1.430977959043743, 0.44208718303333105),
]


def _split_excess_waits(nc):
    """Instructions have one HW sync-wait slot; Tile's slot-release logic
    can emit more. Move the excess onto nofuse NOPs just before the
    instruction on the same engine."""
    import concourse.mybir as mybir

    max_waits = 1
    n_nops = 0
    for fn in nc.m.functions:
        for bb in fn.blocks:
            out = []
            for inst in bb.instructions:
                si = inst.sync_info
                if si is not None and len(si.on_wait) > max_waits:
                    waits = list(si.on_wait)
                    excess, keep = waits[:-max_waits], waits[-max_waits:]
                    while excess:
                        chunk, excess = excess[:max_waits], excess[max_waits:]
                        nop = mybir.InstNoOp(
                            name=f"{inst.name}-wsplit{n_nops}",
                            engine=inst.engine,
                            sync_info=mybir.SyncInfo(on_wait=chunk, on_update=[]),
                            bass_nofuse=True,
                        )
                        n_nops += 1
                        nc.inst_map[nop.name] = nop
                        out.append(nop)
                    inst.sync_info = mybir.SyncInfo(
                        on_wait=keep, on_update=list(si.on_update)
                    )
                out.append(inst)
            bb.instructions[:] = out
    return n_nops


def build_bass(b_shard=B_SHARD):
    import concourse.bass as bass
    import concourse.mybir as mybir
    import concourse.tile as tile

    f32 = mybir.dt.float32
    bf16 = mybir.dt.bfloat16
    Alu = mybir.AluOpType

    K = len(SCHED)
    nblk = b_shard // G
    nc = bass.Bass(name="reeig")
    x = nc.dram_tensor("x", [b_shard, N, N], bf16, kind="ExternalInput")
    out = nc.dram_tensor("out", [b_shard, N, N], bf16, kind="ExternalOutput")

    QUAD = ((0, (0, 0)), (64, (64, 64)))

    with tile.TileContext(nc) as tc:
        with (
            tc.tile_pool(name="const", bufs=1) as cpool,
            tc.tile_pool(name="data", bufs=ILEAVE + 3) as dpool,
            tc.tile_pool(name="xin", bufs=NSLOT) as xpool,
            tc.tile_pool(name="psum", bufs=8, space="PSUM") as ppool,
        ):
            # Block-diagonal X weight slots: one big persistent tile,
            # manually rotated; off-diagonal blocks zeroed once (input DMAs
            # only touch diagonal blocks), so every [128, j, 128] slice
            # stays diag(X_m1, X_m2).
            ablk = cpool.tile([128, NSLOT, GH, 2 * N], bf16, tag="ablk")

            at_tiles = {}

            def issue_load(b):
                if b >= nblk or b in at_tiles:
                    return
                m0 = b * G
                at = xpool.tile([128, GH, N], bf16, tag="X")
                s = b % NSLOT
                nc.sync.dma_start(
                    ablk[0:64, s, :, 0:N],
                    x[m0 : m0 + GH].rearrange("g r c -> r g c"),
                )
                nc.sync.dma_start(
                    ablk[64:128, s, :, N : 2 * N],
                    x[m0 + GH : m0 + G].rearrange("g r c -> r g c"),
                )
                nc.sync.dma_start(
                    at[0:64], x[m0 : m0 + GH].rearrange("g r c -> r g c")
                )
                nc.sync.dma_start(
                    at[64:128], x[m0 + GH : m0 + G].rearrange("g r c -> r g c")
                )
                at_tiles[b] = at

            GHH = GH // 2  # pairs per psum half-tile (1 psum bank each)
            psum_ctr = [0]

            def psum_pair():
                psum_ctr[0] += 1
                n = psum_ctr[0]
                return (ppool.tile([128, GHH, N], f32, tag="PS",
                                   name=f"ps{n}a"),
                        ppool.tile([128, GHH, N], f32, tag="PS",
                                   name=f"ps{n}b"))

            def packed_mm(dst2, rhs_t, slot):
                for j in range(GH):
                    nc.tensor.matmul(
                        dst2[j // GHH][:, j % GHH],
                        lhsT=ablk[:, slot, j],
                        rhs=rhs_t[:, j],
                        start=True, stop=True,
                    )

            def quad_mm(dst2, lhs_t, rhs_t):
                for j in range(GH):
                    for lo, tp in QUAD:
                        nc.tensor.matmul(
                            dst2[j // GHH][lo : lo + 64, j % GHH],
                            lhsT=lhs_t[lo : lo + 64, j],
                            rhs=rhs_t[lo : lo + 64, j],
                            start=True, stop=True, tile_position=tp,
                        )

            def halves(t):
                return ((t[:, 0:GHH], 0), (t[:, GHH:GH], 1))

            # zero the ablk slots needed first, then interleave the rest
            # with the initial prefetch loads
            for s in range(ILEAVE):
                nc.gpsimd.memset(ablk[:, s], 0.0)
            for b in range(ILEAVE):
                issue_load(b)
            for s in range(ILEAVE, NSLOT):
                nc.gpsimd.memset(ablk[:, s], 0.0)
            for b in range(ILEAVE, PF_WAVES * ILEAVE):
                issue_load(b)
            for bp in range(0, nblk, ILEAVE):
                blocks = [b for b in range(bp, min(bp + ILEAVE, nblk))]
                pf = [bp + PF_WAVES * ILEAVE + i for i in range(ILEAVE)]
                st = {}
                for b in blocks:
                    st[b] = {"at": at_tiles.pop(b)}

                for k, (ca, cb) in enumerate(SCHED):
                    g = C / 2 if k == K - 1 else 1.0
                    ys = 1.0 / S**3 if k == 0 else 1.0
                    ps = 1.0 / S if k == 0 else 1.0
                    for i, b in enumerate(blocks):
                        s = st[b]
                        src_t = s["at"] if k == 0 else s["pt"]
                        yt = psum_pair()
                        if k == 0:
                            packed_mm(yt, src_t, b % NSLOT)
                        else:
                            quad_mm(yt, src_t, src_t)
                        s["yt"] = yt
                        if i < len(pf) and i % K == k:
                            issue_load(pf[i])
                    for b in blocks:
                        s = st[b]
                        ypt = dpool.tile([128, GH, N], bf16, tag="Yp")
                        for sl, h in halves(ypt):
                            nc.scalar.mul(sl, s["yt"][h][:], -cb * g * ys)
                        s["ypt"] = ypt
                    for b in blocks:
                        s = st[b]
                        src_t = s["at"] if k == 0 else s["pt"]
                        zt = psum_pair()
                        if k == 0:
                            packed_mm(zt, s["ypt"], b % NSLOT)
                        else:
                            quad_mm(zt, src_t, s["ypt"])
                        s["zt"] = zt
                    for b in blocks:
                        s = st[b]
                        src_t = s["at"] if k == 0 else s["pt"]
                        pt = dpool.tile([128, GH, N], bf16, tag="P")
                        for sl, h in halves(pt):
                            nc.vector.scalar_tensor_tensor(
                                out=sl, in0=halves(src_t)[h][0],
                                scalar=ca * g * ps,
                                in1=s["zt"][h][:], op0=Alu.mult, op1=Alu.add,
                            )
                        s["pt"] = pt

                for b in blocks:
                    s = st[b]
                    wt = psum_pair()
                    packed_mm(wt, s["pt"], b % NSLOT)
                    s["wt"] = wt
                    rt = dpool.tile([128, GH, N], bf16, tag="R")
                    # W = (C*s/2) A P~ only; the 0.5*X half of rec is added
                    # on the host in fp32 (cheaper here and more accurate).
                    # Alternate the psum drains between Act and DVE so
                    # neither queue gates the next wave's Y0 psum reuse.
                    for sl, h in halves(rt):
                        if (b + h) % 2 == 0:
                            nc.scalar.mul(sl, s["wt"][h][:], 1.0)
                        else:
                            nc.vector.tensor_scalar_mul(sl, s["wt"][h][:], 1.0)
                    m0 = b * G
                    nc.gpsimd.dma_start(
                        out[m0 : m0 + GH].rearrange("g r c -> r g c"), rt[0:64]
                    )
                    nc.gpsimd.dma_start(
                        out[m0 + GH : m0 + G].rearrange("g r c -> r g c"),
                        rt[64:128],
                    )

    _split_excess_waits(nc)
    return nc


_CACHE = {}


def run(x: np.ndarray, **spmd_kwargs):
    import ml_dtypes
    from concourse.bass_utils import run_bass_kernel_spmd

    assert x.shape == (B, N, N) and x.dtype == np.float32
    if "nc" not in _CACHE:
        _CACHE["nc"] = build_bass()
    nc = _CACHE["nc"]
    xb = x.astype(ml_dtypes.bfloat16)
    shards = xb.reshape(N_CORES, B_SHARD, N, N)
    in_maps = [{"x": np.ascontiguousarray(shards[i])} for i in range(N_CORES)]
    return run_bass_kernel_spmd(
        nc, in_maps, core_ids=list(range(N_CORES)), **spmd_kwargs
    )


def kernel(x: np.ndarray) -> np.ndarray:
    x = np.ascontiguousarray(np.asarray(x), dtype=np.float32)
    res = run(x)
    w = np.concatenate(
        [r["out"].astype(np.float32) for r in res.results], axis=0
    )
    # rec = 0.5*X + W, symmetrized (W is symmetric up to bf16 matmul noise;
    # averaging with the transpose halves it)
    out = 0.5 * x + 0.5 * (w + w.transpose(0, 2, 1))
    return out.astype(np.float32)
